# revision 1
# baseline (speedup 1.0000x reference)
"""Trainium2 Bass kernel for causal multi-head self-attention.

Problem (hardcoded):
    x:      [2, 2048, 1024] f32
    W_qkv:  [1024, 3072] f32   (cols: [q | k | v], each 1024 = 16 heads x 64)
    b_qkv:  [3072] f32
    W_proj: [1024, 1024] f32
    b_proj: [1024] f32
    out:    [2, 2048, 1024] f32

Sharding over 8 NeuronCores: data parallel on batch (2) x tensor parallel on
heads (4 quads of 4 heads). Core c handles batch c//4, heads [4*(c%4), 4*(c%4)+4).
Each core computes its heads' q/k/v projections, causal-softmax attention, and a
partial output projection (its heads' rows of W_proj). Host gather sums the 4
partials per batch and adds b_proj (the standard tensor-parallel unshard).

On-core dataflow (all matmuls in float32r: 1 cycle/row for N>=256):
  - qkT [512, S]   = W_qk^T @ x^T   (transposed layout: partitions = qkv-col)
  - v   [S, 256]   = x @ W_v, augmented with a ones column per head -> [.,65]
  - scoresT[sk,sq] = kT.T @ qT per head, causal-lower blocks only
  - expT = ACT exp(0.125 * scoresT) (no max-subtraction: |scores/8| <= ~3)
  - psum_av[65,sq] = [V|1]^T @ expT  -> rows 0:64 unnormalized attnT, row 64 sums
  - normalize via reciprocal + DRAM-bounce partition-broadcast + multiply
  - y_partial[S, 1024] = attnT^T @ W_proj_slice  (K=64 per head)
"""

import os
import sys

for _p in ("/opt/trn_rl_repo", "/root/.axon_site/_ro/trn_rl_repo"):
    if os.path.isdir(_p) and _p not in sys.path:
        sys.path.append(_p)

import numpy as np

import concourse.bass as bass
import concourse.mybir as mybir
import concourse.tile as tile
from concourse import library_config

F32 = mybir.dt.float32
F32R = mybir.dt.float32r
AFT = mybir.ActivationFunctionType

B, S, D, H, HD = 2, 2048, 1024, 16, 64
NCORES = 8
NH = 4  # heads per core
SCALE = 1.0 / 8.0  # 1/sqrt(64)


class SplitWaitTileContext(tile.TileContext):
    """This container's walrus rejects >1 sync wait per instruction
    ("Too many sync wait commands"). Split extra waits onto preceding
    same-engine NoOps before the final block lowering."""

    def _lower_ordered_insts(self, ordered):
        for bb_name, insts in list(ordered.items()):
            new = []
            for inst in insts:
                si = inst.sync_info
                if si is not None and si.on_wait and len(si.on_wait) > 1:
                    waits = list(si.on_wait)
                    for w in waits[:-1]:
                        nop = mybir.InstNoOp(
                            name=f"nopw-{self.nc.get_next_instruction_name()}"
                        )
                        nop.engine = inst.engine
                        nop.sync_info = mybir.SyncInfo(on_wait=[w], on_update=[])
                        new.append(nop)
                    inst.sync_info = mybir.SyncInfo(
                        on_wait=[waits[-1]], on_update=list(si.on_update or [])
                    )
                new.append(inst)
            ordered[bb_name] = new
        return super()._lower_ordered_insts(ordered)

    def _drain_and_barrier(self, tick_clock, wait_clock):
        from concourse.vector_clock import ScopedClock

        drain_inst = self.nc.sync.drain()
        wait_clock.add_sem_waits(
            drain_inst.ins, ScopedClock({None: tick_clock.global_clock})
        )
        si = drain_inst.ins.sync_info
        if si is not None and si.on_wait and len(si.on_wait) > 1:
            waits = list(si.on_wait)
            drain_inst.ins.sync_info = mybir.SyncInfo(
                on_wait=[waits[0]], on_update=list(si.on_update or [])
            )
            for w in waits[1:]:
                nop = self.nc.sync.nop(nofuse=True)
                nop.ins.sync_info = mybir.SyncInfo(on_wait=[w], on_update=[])

        self.nc.all_engine_barrier()
        assert self.sems is not None
        popped = self.nc._tile_sem_poison_stack.pop()
        assert popped is self._sem_poison
        self.nc.clear_and_free_semaphores(list(self.sems.allocated().values()))
        self.nc.all_engine_barrier()


def build_nc(S=S, D=D, NH=NH, dbg=False, reps=1):
    """Build the single-core SPMD program. Parameterized for small-sim testing."""
    KD = D // 128        # k-chunks of the D contraction
    NM = NH * 2 * 64 // 128   # qk M-tiles (q chunks then k chunks)
    NMQ = NM // 2
    SQB = S // 512       # sq blocks of 512
    NSK = S // 128       # sk tiles of 128
    NB = min(512, D)     # proj output column block size
    ND = D // NB         # proj output column blocks
    NHJ = NH * SQB

    nc = bass.Bass("TRN2", target_bir_lowering=False, debug=False)

    xT_d = nc.dram_tensor("xT", [D, S], F32R, kind="ExternalInput").ap()
    wqk_d = nc.dram_tensor("wqk", [D, NM * 128], F32R, kind="ExternalInput").ap()
    wv_d = nc.dram_tensor("wv", [D, NH * 64], F32R, kind="ExternalInput").ap()
    bqk_d = nc.dram_tensor("bqk", [NM, 128], F32, kind="ExternalInput").ap()
    bvbc_d = nc.dram_tensor("bvbc", [128, NH * 64], F32, kind="ExternalInput").ap()
    wproj_d = nc.dram_tensor("wproj", [NH * 64, D], F32R, kind="ExternalInput").ap()
    mask_d = nc.dram_tensor("mask4", [128, 128], F32R, kind="ExternalInput").ap()
    y_d = nc.dram_tensor("y", [S, D], F32, kind="ExternalOutput").ap()
    scratch_d = nc.dram_tensor("scratch", [1, NHJ, 512], F32R).ap()

    with SplitWaitTileContext(nc) as tc:
        with (
            nc.allow_low_precision(reason="fp32r feeds PE; fp32 accum in PSUM"),
            tc.tile_pool(name="stream", bufs=2) as p_stream,
            tc.tile_pool(name="attnp", bufs=1) as p_attn,
            tc.tile_pool(name="wpool", bufs=1) as p_w,
            tc.tile_pool(name="qkt", bufs=1) as p_qkt,
            tc.tile_pool(name="vaug", bufs=1) as p_vaug,
            tc.tile_pool(name="expp", bufs=4) as p_exp,
            tc.tile_pool(name="dtmpp", bufs=4) as p_dtmp,
            tc.tile_pool(name="ypool", bufs=4) as p_y,
            tc.tile_pool(name="pmisc", bufs=2, space="PSUM") as p_misc,
            tc.tile_pool(name="ps", bufs=2, space="PSUM") as p_s,
            tc.tile_pool(name="pav", bufs=2, space="PSUM") as p_av,
        ):
          for _rep in range(reps):
            # PE warmup: junk matmuls on a memset tile keep the systolic array
            # ramped (HAM K=8/8) while the input DMAs land
            ones_sb = p_w.tile([128, 64], F32R, tag="ones")
            nc.vector.memset(ones_sb[:, :].bitcast(F32), 1.0)
            # preload the exp table set (~2.7us) in the startup window so the
            # first real softmax exp doesn't pay it on the critical chain
            expwarm = p_w.tile([1, 1], F32, tag="expwarm")
            nc.scalar.activation(
                expwarm[:, :], ones_sb[0:1, 0:1], AFT.Exp, scale=SCALE
            )
            warm_ps = p_av.tile([64, 64], F32, tag="av")
            for _w in range(40):
                nc.tensor.matmul(
                    warm_ps[:, :],
                    lhsT=ones_sb[:, :],
                    rhs=ones_sb[:, :],
                    start=True,
                    stop=True,
                )

            # biases are tiny but gate the qkT adds: issue them first
            bqk_sb = p_w.tile([128, NM], F32, tag="bqk")
            nc.sync.dma_start(out=bqk_sb[:, :], in_=bqk_d.rearrange("m p -> p m"))
            bvbc_sb = p_w.tile([128, NH * 64], F32, tag="bvbc")
            nc.sync.dma_start(out=bvbc_sb[:, :], in_=bvbc_d[:, :])

            # xT column 0 + qk weights first: they gate the first matmuls
            xT_src0 = xT_d.rearrange("(c p) s -> p c s", p=128)
            xs0 = p_stream.tile([128, KD, 512], F32R, tag="xs")
            for k in range(KD):
                nc.sync.dma_start(out=xs0[:, k, :], in_=xT_src0[:, k, 0:512])

            wqk_sb = p_w.tile([128, KD, NM * 128], F32R, tag="wqk")
            wqk_src = wqk_d.rearrange("(c p) n -> p c n", p=128)
            for k in range(KD):
                nc.sync.dma_start(out=wqk_sb[:, k, :], in_=wqk_src[:, k, :])

            wv_sb = p_w.tile([128, KD, NH * 64], F32R, tag="wv")
            wv_src = wv_d.rearrange("(c p) n -> p c n", p=128)
            for k in range(KD):
                nc.sync.dma_start(out=wv_sb[:, k, :], in_=wv_src[:, k, :])

            wproj_sb = p_w.tile([64, NH, D], F32R, tag="wproj")
            nc.sync.dma_start(
                out=wproj_sb[:, :, :],
                in_=wproj_d.rearrange("(h cc) n -> cc h n", cc=64),
            )

            mask_sb = p_w.tile([128, 128], F32R, tag="mask")
            nc.sync.dma_start(out=mask_sb[:, :], in_=mask_d[:, :])

            qkT_sb = p_qkt.tile([128, NM, S], F32R, tag="qkt")
            v_aug = p_vaug.tile([128, NSK, NH, 65], F32R, tag="vaug")
            nc.vector.memset(v_aug[:, :, :, 64:65].bitcast(F32), 1.0)
            # attn_st rows 0:64 = unnormalized attnT, row 64 = softmax sums;
            # index hj = j * NH + h
            attn_st = p_attn.tile([65, NHJ, 512], F32R, tag="attn")

            xT_src = xT_d.rearrange("(c p) s -> p c s", p=128)

            def load_xs(j):
                xs = p_stream.tile([128, KD, 512], F32R, tag="xs")
                for k in range(KD):
                    nc.sync.dma_start(
                        out=xs[:, k, :],
                        in_=xT_src[:, k, j * 512:(j + 1) * 512],
                    )
                return xs

            def qk_part(j, xs, mp):
                ps_qk = p_misc.tile([128, 512], F32, tag="m")
                for k in range(KD):
                    nc.tensor.matmul(
                        ps_qk[:, :],
                        lhsT=wqk_sb[:, k, mp * 128:(mp + 1) * 128],
                        rhs=xs[:, k, :],
                        start=(k == 0),
                        stop=(k == KD - 1),
                    )
                nc.vector.tensor_scalar_add(
                    qkT_sb[:, mp, j * 512:(j + 1) * 512],
                    ps_qk[:, :],
                    bqk_sb[:, mp:mp + 1],
                )

            def v_part(j, xs, m):
                ps_v = p_misc.tile([128, NH * 64], F32, tag="m")
                for k in range(KD):
                    nc.tensor.matmul(
                        ps_v[:, :],
                        lhsT=xs[:, k, (m % 4) * 128:(m % 4) * 128 + 128],
                        rhs=wv_sb[:, k, :],
                        start=(k == 0),
                        stop=(k == KD - 1),
                    )
                nc.vector.tensor_add(
                    v_aug[:, m, :, 0:64],
                    ps_v[:, :].rearrange("p (h c) -> p h c", c=64),
                    bvbc_sb[:, :].rearrange("p (h c) -> p h c", c=64),
                )

            def qkv_parts(j, xs):
                parts = []
                for mp in range(NM):
                    parts.append(lambda mp=mp: qk_part(j, xs, mp))
                for m in range(4 * j, 4 * j + 4):
                    parts.append(lambda m=m: v_part(j, xs, m))
                return parts

            def attention_block(j, fillers=()):
                fillers = list(fillers)
                for h in range(NH):
                    qT = qkT_sb[64 * (h % 2):64 * (h % 2) + 64, h // 2, :]
                    kT = qkT_sb[64 * (h % 2):64 * (h % 2) + 64, NMQ + h // 2, :]
                    ps_av = p_av.tile([65, 512], F32, tag="av")
                    npair = 2 * (j + 1)

                    def noff(i):
                        # causal column truncation: sk-tile i only touches
                        # sq >= 128*(i-4j); keep N >= 256 for the fp32r
                        # 1-cycle/row fast path
                        mb = i - 4 * j
                        return 0 if mb <= 0 else min(128 * mb, 256)

                    def emit_scores(g):
                        ps = p_s.tile([128, 2, 512], F32, tag="s")
                        for b in range(2):
                            i = 2 * g + b
                            no = noff(i)
                            nc.tensor.matmul(
                                ps[:, b, no:512],
                                lhsT=kT[:, i * 128:(i + 1) * 128],
                                rhs=qT[:, j * 512 + no:(j + 1) * 512],
                                start=True,
                                stop=True,
                            )
                        return ps

                    sc_next = emit_scores(0)
                    for g in range(npair):
                        ps_sc = sc_next
                        # 1-deep software pipeline: next group's scores are
                        # emitted before this group's AV so PE runs them
                        # while ACT computes this group's exp
                        if g + 1 < npair:
                            sc_next = emit_scores(g + 1)
                        exp_t = p_exp.tile([128, 2, 512], F32R, tag="exp")
                        if g >= 2 * j:
                            # zero the causally-dead columns up front so the
                            # fills run concurrent with scores/exp instead of
                            # on the exp->AV edge (cols the exp call will
                            # overwrite are re-zeroed after it below)
                            for b in range(2):
                                mb = 2 * g + b - 4 * j
                                if mb > 0:
                                    nc.vector.memset(
                                        exp_t[:, b, 0:min(128 * mb, 256)].bitcast(F32),
                                        0.0,
                                    )
                        if g == 2 * j + 1:
                            # both tiles dead below column 256 -> smaller exp
                            nc.scalar.activation(
                                exp_t[:, :, 256:512],
                                ps_sc[:, :, 256:512],
                                AFT.Exp,
                                scale=SCALE,
                            )
                            # mb=3: cols 256:384 are dead but were just
                            # overwritten by the batched exp; re-zero them
                            nc.vector.memset(
                                exp_t[:, 1, 256:384].bitcast(F32), 0.0
                            )
                        elif g == 2 * j:
                            # b=0 full; b=1 only columns >= 128 were computed
                            nc.scalar.activation(
                                exp_t[:, 0, :], ps_sc[:, 0, :], AFT.Exp, scale=SCALE
                            )
                            nc.scalar.activation(
                                exp_t[:, 1, 128:512],
                                ps_sc[:, 1, 128:512],
                                AFT.Exp,
                                scale=SCALE,
                            )
                        else:
                            nc.scalar.activation(
                                exp_t[:, :, :], ps_sc[:, :, :], AFT.Exp, scale=SCALE
                            )
                        if g >= 2 * j:  # diagonal pair: causal mask inside blocks
                            for b in range(2):
                                mb = 2 * g + b - 4 * j
                                nc.vector.tensor_mul(
                                    exp_t[:, b, 128 * mb:128 * mb + 128],
                                    exp_t[:, b, 128 * mb:128 * mb + 128],
                                    mask_sb[:, :],
                                )
                        for b in range(2):
                            i = 2 * g + b
                            no = noff(i)
                            nc.tensor.matmul(
                                ps_av[:, no:512],
                                lhsT=v_aug[:, i, h, :],
                                rhs=exp_t[:, b, no:512],
                                start=(g == 0 and b == 0),
                                stop=(g == npair - 1 and b == 1),
                            )
                    nc.vector.tensor_copy(attn_st[:, j * NH + h, :], ps_av[:, :])
                    if h % 2 == 1:
                        # normalize this head pair (DVE + GpSimd, off PE path)
                        idx = j * NH + h - 1
                        nc.vector.reciprocal(
                            attn_st[64:65, idx:idx + 2, :],
                            attn_st[64:65, idx:idx + 2, :],
                        )
                        if j == SQB - 1 and h == NH - 1:
                            # kernel tail: PE is idle here, so broadcast the
                            # recip row via K=1 outer products instead of the
                            # (higher-latency) DRAM-bounce DMA
                            for t in range(2):
                                ps_d = p_misc.tile([64, 512], F32, tag="m")
                                nc.tensor.matmul(
                                    ps_d[:, :],
                                    lhsT=ones_sb[64:65, :],
                                    rhs=attn_st[64:65, idx + t, :],
                                    start=True,
                                    stop=True,
                                )
                                nc.vector.tensor_mul(
                                    attn_st[0:64, idx + t, :],
                                    attn_st[0:64, idx + t, :],
                                    ps_d[:, :],
                                )
                        else:
                            nc.sync.dma_start(
                                out=scratch_d[:, idx:idx + 2, :],
                                in_=attn_st[64:65, idx:idx + 2, :],
                            )
                            dtmp = p_dtmp.tile([64, 2, 512], F32R, tag="dtmp")
                            srcp = scratch_d[0, idx:idx + 2, :]
                            nc.sync.dma_start(
                                out=dtmp[:, :, :],
                                in_=bass.AP(
                                    tensor=srcp.tensor,
                                    offset=srcp.offset,
                                    ap=[[0, 64]] + list(srcp.ap),
                                ),
                            )
                            nc.vector.tensor_mul(
                                attn_st[0:64, idx:idx + 2, :],
                                attn_st[0:64, idx:idx + 2, :],
                                dtmp[:, :, :],
                            )
                    # drain PE filler work into the ACT-paced stretch;
                    # ACT lag builds late in the block, so weight fillers there
                    if h >= 1:
                        take = max(1, (len(fillers) + NH - 1 - h) // (NH - h))
                        for _ in range(take):
                            if fillers:
                                fillers.pop(0)()
                for f in fillers:
                    f()

            def proj_part(j, m):
                o = (m % 4) * 128
                for n in range(ND):
                    ps_y = p_misc.tile([128, NB], F32, tag="m")
                    for h in range(NH):
                        nc.tensor.matmul(
                            ps_y[:, :],
                            lhsT=attn_st[0:64, j * NH + h, o:o + 128],
                            rhs=wproj_sb[:, h, n * NB:(n + 1) * NB],
                            start=(h == 0),
                            stop=(h == NH - 1),
                        )
                    y_sb = p_y.tile([128, NB], F32, tag="y")
                    nc.vector.tensor_copy(y_sb[:, :], ps_y[:, :])
                    nc.sync.dma_start(
                        out=y_d[m * 128:(m + 1) * 128, n * NB:(n + 1) * NB],
                        in_=y_sb[:, :],
                    )

            def proj_parts(j):
                return [
                    (lambda m=m: proj_part(j, m)) for m in range(j * 4, j * 4 + 4)
                ]

            # j=0 prologue: qkv computed up front (xs0 DMA'd first, see
            # above). The qk parts come first (their weights land first);
            # junk matmuls bridge the wv-DMA wait so PE stays busy/warm.
            parts0 = qkv_parts(0, xs0)
            for part in parts0[:NM]:
                part()
            for _w in range(40):
                nc.tensor.matmul(
                    warm_ps[:, :],
                    lhsT=ones_sb[:, :],
                    rhs=ones_sb[:, :],
                    start=True,
                    stop=True,
                )
            for part in parts0[NM:]:
                part()
            xs_next = load_xs(1) if SQB > 1 else None
            for j in range(SQB):
                fillers = []
                if j + 1 < SQB:
                    fillers += qkv_parts(j + 1, xs_next)
                    xs_after = load_xs(j + 2) if j + 2 < SQB else None
                else:
                    xs_after = None
                if j >= 1:
                    fillers += proj_parts(j - 1)
                attention_block(j, fillers)
                xs_next = xs_after
            for part in proj_parts(SQB - 1):
                part()

    return nc


def make_mask4():
    p = np.arange(128)[:, None]
    f = np.arange(128)[None, :]
    return (f >= p).astype(np.float32).copy()  # [128, 128] lower-tri in T layout


def make_in_maps(x, W_qkv, b_qkv, W_proj):
    """Per-core input dicts for the full-size problem."""
    mask4 = make_mask4()
    in_maps = []
    for c in range(NCORES):
        b, q = c // 4, c % 4
        cq = slice(256 * q, 256 * q + 256)
        wqk = np.concatenate([W_qkv[:, cq], W_qkv[:, 1024:2048][:, cq]], axis=1)
        wv = W_qkv[:, 2048:3072][:, cq]
        bqk = np.concatenate([b_qkv[cq], b_qkv[1024:2048][cq]]).reshape(4, 128)
        bvbc = np.broadcast_to(b_qkv[2048:3072][cq], (128, 256))
        in_maps.append(
            {
                "xT": np.ascontiguousarray(x[b].T),
                "wqk": np.ascontiguousarray(wqk),
                "wv": np.ascontiguousarray(wv),
                "bqk": np.ascontiguousarray(bqk),
                "bvbc": np.ascontiguousarray(bvbc),
                "wproj": np.ascontiguousarray(W_proj[cq, :]),
                "mask4": mask4,
            }
        )
    return in_maps


_NC_CACHE = {}


def _get_nc():
    if "nc" not in _NC_CACHE:
        _NC_CACHE["nc"] = build_nc()
    return _NC_CACHE["nc"]


def run_on_hw(x, W_qkv, b_qkv, W_proj, b_proj, trace=False, **trace_kw):
    from concourse.bass_utils import run_bass_kernel_spmd

    in_maps = make_in_maps(x, W_qkv, b_qkv, W_proj)
    res = run_bass_kernel_spmd(
        _get_nc(), in_maps, core_ids=list(range(NCORES)), trace=trace, **trace_kw
    )
    out = np.empty((B, S, D), dtype=np.float32)
    for b in range(B):
        acc = res.results[4 * b]["y"].astype(np.float32).copy()
        for q in range(1, 4):
            acc += res.results[4 * b + q]["y"]
        out[b] = acc + b_proj[None, :]
    return out, res


def make_runner(nc, n_cores=NCORES):
    """Cached-jit runner mirroring bass2jax.run_bass_via_pjrt, but reusable:
    inputs stay on device, outputs chain into the next call's donated buffers."""
    import jax
    from jax.sharding import Mesh, PartitionSpec
    from jax.experimental.shard_map import shard_map
    from concourse import bass2jax
    from concourse import mybir as mb

    bass2jax.install_neuronx_cc_hook()
    partition_name = (
        nc.partition_id_tensor.name if nc.partition_id_tensor else None
    )
    in_names, out_names, out_avals, zero_outs = [], [], [], []
    for alloc in nc.m.functions[0].allocations:
        if not isinstance(alloc, mb.MemoryLocationSet):
            continue
        name = alloc.memorylocations[0].name
        if alloc.kind == "ExternalInput":
            if name != partition_name:
                in_names.append(name)
        elif alloc.kind == "ExternalOutput":
            import concourse.dt as cdt

            npdt = cdt.dt.np(alloc.dtype)
            out_avals.append(
                jax.core.ShapedArray(tuple(alloc.tensor_shape), npdt)
            )
            out_names.append(name)
            zero_outs.append(np.zeros(tuple(alloc.tensor_shape), npdt))
    n_params = len(in_names)
    all_in_names = tuple(
        in_names + out_names + ([partition_name] if partition_name else [])
    )

    def _body(*args):
        operands = list(args)
        if partition_name is not None:
            operands.append(bass2jax.partition_id_tensor())
        outs = bass2jax._bass_exec_p.bind(
            *operands,
            out_avals=tuple(out_avals),
            in_names=all_in_names,
            out_names=tuple(out_names),
            lowering_input_output_aliases=(),
            sim_require_finite=True,
            sim_require_nnan=True,
            nc=nc,
        )
        return tuple(outs)

    devices = jax.devices()[:n_cores]
    mesh = Mesh(np.asarray(devices), ("core",))
    donate = tuple(range(n_params, n_params + len(out_names)))
    fn = jax.jit(
        shard_map(
            _body,
            mesh=mesh,
            in_specs=(PartitionSpec("core"),) * (n_params + len(out_names)),
            out_specs=(PartitionSpec("core"),) * len(out_names),
            check_rep=False,
        ),
        donate_argnums=donate,
        keep_unused=True,
    )
    return fn, in_names, out_names, zero_outs, mesh


def run_timed(reps_list=(1, 4), iters=6):
    """Estimate per-kernel-execution time from the slope of wall time vs
    in-NEFF repetition count (cancels RPC/jit/transfer overhead)."""
    import jax
    import time
    from jax.sharding import NamedSharding, PartitionSpec

    d = np.load("/tmp/ref_mhsa.npz")
    in_maps = make_in_maps(d["x"], d["W_qkv"], d["b_qkv"], d["W_proj"])
    results = {}
    for reps in reps_list:
        nc = build_nc(reps=reps)
        fn, in_names, out_names, zero_outs, mesh = make_runner(nc)
        sh = NamedSharding(mesh, PartitionSpec("core"))
        dev_ins = [
            jax.device_put(
                np.concatenate([in_maps[c][n] for c in range(NCORES)], axis=0), sh
            )
            for n in in_names
        ]
        outs = tuple(
            jax.device_put(
                np.zeros((NCORES * z.shape[0], *z.shape[1:]), z.dtype), sh
            )
            for z in zero_outs
        )
        times = []
        for it in range(iters):
            t0 = time.perf_counter()
            outs = fn(*dev_ins, *outs)
            jax.block_until_ready(outs)
            times.append(time.perf_counter() - t0)
        results[reps] = (min(times[1:]), times)
        print(f"reps={reps}: min {min(times[1:])*1e3:.3f} ms  all {[f'{t*1e3:.1f}' for t in times]}")
    if len(reps_list) >= 2:
        r0, r1 = reps_list[0], reps_list[-1]
        slope = (results[r1][0] - results[r0][0]) / (r1 - r0)
        print(f"HW exec time (slope): {slope*1e9:.0f} ns")
        return slope * 1e9
    return None


def kernel(x, W_qkv, b_qkv, W_proj, b_proj):
    x = np.asarray(x, dtype=np.float32)
    W_qkv = np.asarray(W_qkv, dtype=np.float32)
    b_qkv = np.asarray(b_qkv, dtype=np.float32)
    W_proj = np.asarray(W_proj, dtype=np.float32)
    b_proj = np.asarray(b_proj, dtype=np.float32)
    out, _ = run_on_hw(x, W_qkv, b_qkv, W_proj, b_proj, trace=False)
    return out



# revision 4
# speedup vs baseline: 1.0944x; 1.0944x over previous
"""Trainium2 Bass kernel for causal multi-head self-attention.

Problem (hardcoded):
    x:      [2, 2048, 1024] f32
    W_qkv:  [1024, 3072] f32   (cols: [q | k | v], each 1024 = 16 heads x 64)
    b_qkv:  [3072] f32
    W_proj: [1024, 1024] f32
    b_proj: [1024] f32
    out:    [2, 2048, 1024] f32

Sharding over 8 NeuronCores: data parallel on batch (2) x tensor parallel on
heads (4 quads of 4 heads). Core c handles batch c//4, heads [4*(c%4), 4*(c%4)+4).
Each core computes its heads' q/k/v projections, causal-softmax attention, and a
partial output projection (its heads' rows of W_proj). Host gather sums the 4
bf16 partials per batch in f32 and adds b_proj.

On-core dataflow (bf16 operands, f32 PSUM accumulation):
  - qkT [512, S]   = W_qk^T @ x^T   (partitions = qkv-col; 4 M-tiles of 128)
  - v_aug [S,4,65] = x @ W_v + ones column per head (softmax denominator)
  - scoresT[sk,sq] = kT.T @ qT per head, exact 128-granular causal truncation
  - expT = ACT exp(0.125 * scores) -> bf16 (no max-subtraction: |s/8| small)
  - diagonal tiles masked on GpSimd (Pool) with a [128,128] triangular mask
  - AV transposed: ps_attn[sq 128, 4 chunks, 65] += expT_chunk^T @ v_aug
    (N=65 per accumulation step: half the PE cycles of the [65, sq] form)
  - normalize per-partition (sq) via DVE reciprocal + broadcast multiply
  - head pair packed side by side [sq, 2*64] -> XBAR DMA transpose -> [128, sq]
  - y tile [sq 128, 512] = sum over 2 pairs: attn_T^T @ W_proj_pair (K=128)
"""

import os
import sys

for _p in ("/opt/trn_rl_repo", "/root/.axon_site/_ro/trn_rl_repo"):
    if os.path.isdir(_p) and _p not in sys.path:
        sys.path.append(_p)

import numpy as np

import concourse.bass as bass
import concourse.mybir as mybir
import concourse.tile as tile
from concourse import library_config

F32 = mybir.dt.float32
BF16 = mybir.dt.bfloat16
AFT = mybir.ActivationFunctionType

B, S, D, H, HD = 2, 2048, 1024, 16, 64
NCORES = 8
NH = 4  # heads per core
SCALE = 1.0 / 8.0  # 1/sqrt(64)


class SplitWaitTileContext(tile.TileContext):
    """This container's walrus rejects >1 sync wait per instruction
    ("Too many sync wait commands"). Split extra waits onto preceding
    same-engine NoOps before the final block lowering."""

    def _lower_ordered_insts(self, ordered):
        for bb_name, insts in list(ordered.items()):
            new = []
            for inst in insts:
                si = inst.sync_info
                if si is not None and si.on_wait and len(si.on_wait) > 1:
                    waits = list(si.on_wait)
                    for w in waits[:-1]:
                        nop = mybir.InstNoOp(
                            name=f"nopw-{self.nc.get_next_instruction_name()}"
                        )
                        nop.engine = inst.engine
                        nop.sync_info = mybir.SyncInfo(on_wait=[w], on_update=[])
                        new.append(nop)
                    inst.sync_info = mybir.SyncInfo(
                        on_wait=[waits[-1]], on_update=list(si.on_update or [])
                    )
                new.append(inst)
            ordered[bb_name] = new
        return super()._lower_ordered_insts(ordered)

    def _drain_and_barrier(self, tick_clock, wait_clock):
        from concourse.vector_clock import ScopedClock

        drain_inst = self.nc.sync.drain()
        wait_clock.add_sem_waits(
            drain_inst.ins, ScopedClock({None: tick_clock.global_clock})
        )
        si = drain_inst.ins.sync_info
        if si is not None and si.on_wait and len(si.on_wait) > 1:
            waits = list(si.on_wait)
            drain_inst.ins.sync_info = mybir.SyncInfo(
                on_wait=[waits[0]], on_update=list(si.on_update or [])
            )
            for w in waits[1:]:
                nop = self.nc.sync.nop(nofuse=True)
                nop.ins.sync_info = mybir.SyncInfo(on_wait=[w], on_update=[])

        self.nc.all_engine_barrier()
        assert self.sems is not None
        popped = self.nc._tile_sem_poison_stack.pop()
        assert popped is self._sem_poison
        self.nc.clear_and_free_semaphores(list(self.sems.allocated().values()))
        self.nc.all_engine_barrier()


def build_nc(S=S, D=D, NH=NH, dbg=False, reps=1):
    """Build the single-core SPMD program."""
    KD = D // 128        # k-chunks of the D contraction
    NM = NH              # qk M-tiles: 2 q tiles then 2 k tiles
    NMQ = NM // 2
    SQB = S // 512       # sq blocks of 512
    NSK = S // 128       # sk tiles of 128
    NB = min(512, D)     # proj output column block size
    ND = D // NB         # proj output column blocks
    NPAIR = NH // 2

    nc = bass.Bass("TRN2", target_bir_lowering=False, debug=False)

    xT_d = nc.dram_tensor("xT", [D, S], BF16, kind="ExternalInput").ap()
    wqk_d = nc.dram_tensor("wqk", [D, NM * 128], BF16, kind="ExternalInput").ap()
    wv_d = nc.dram_tensor("wv", [D, NH * 64], BF16, kind="ExternalInput").ap()
    bqk_d = nc.dram_tensor("bqk", [NM, 128], F32, kind="ExternalInput").ap()
    bvbc_d = nc.dram_tensor("bvbc", [128, NH * 64], F32, kind="ExternalInput").ap()
    wproj_d = nc.dram_tensor(
        "wproj", [128, NPAIR, D], BF16, kind="ExternalInput"
    ).ap()
    mask_d = nc.dram_tensor("masku", [128, 128], BF16, kind="ExternalInput").ap()
    y_d = nc.dram_tensor("y", [S, D], BF16, kind="ExternalOutput").ap()

    with SplitWaitTileContext(nc) as tc:
        with (
            nc.allow_low_precision(reason="bf16 operands; fp32 accum in PSUM"),
            tc.tile_pool(name="stream", bufs=2) as p_stream,
            tc.tile_pool(name="wpool", bufs=1) as p_w,
            tc.tile_pool(name="qkt", bufs=1) as p_qkt,
            tc.tile_pool(name="vaug", bufs=1) as p_vaug,
            tc.tile_pool(name="expp", bufs=4) as p_exp,
            tc.tile_pool(name="attnn", bufs=2) as p_attn_n,
            tc.tile_pool(name="attnT", bufs=2) as p_attn_T,
            tc.tile_pool(name="rcp", bufs=2) as p_rc,
            tc.tile_pool(name="ypool", bufs=4) as p_y,
            tc.tile_pool(name="pmisc", bufs=2, space="PSUM") as p_misc,
            tc.tile_pool(name="ps", bufs=2, space="PSUM") as p_s,
            tc.tile_pool(name="pav", bufs=2, space="PSUM") as p_av,
        ):
          for _rep in range(reps):
            # PE warmup: junk matmuls keep the systolic array ramped while
            # the input DMAs land
            ones_sb = p_w.tile([128, 64], BF16, tag="ones")
            nc.vector.memset(ones_sb[:, :], 1.0)
            # preload the exp table set in the startup window
            expwarm = p_w.tile([1, 1], F32, tag="expwarm")
            nc.scalar.activation(
                expwarm[:, :], ones_sb[0:1, 0:1], AFT.Exp, scale=SCALE
            )
            warm_ps = p_av.tile([128, NH, 65], F32, tag="av")
            for _w in range(40):
                nc.tensor.matmul(
                    warm_ps[0:64, 0, 0:64],
                    lhsT=ones_sb[:, :],
                    rhs=ones_sb[:, :],
                    start=True,
                    stop=True,
                )

            # biases are tiny but gate the qkT adds: issue them first
            bqk_sb = p_w.tile([128, NM], F32, tag="bqk")
            nc.sync.dma_start(out=bqk_sb[:, :], in_=bqk_d.rearrange("m p -> p m"))
            bvbc_sb = p_w.tile([128, NH * 64], F32, tag="bvbc")
            nc.sync.dma_start(out=bvbc_sb[:, :], in_=bvbc_d[:, :])

            # xT column 0 + qk weights first: they gate the first matmuls
            xT_src = xT_d.rearrange("(c p) s -> p c s", p=128)
            xs0 = p_stream.tile([128, KD, 512], BF16, tag="xs")
            for k in range(KD):
                nc.sync.dma_start(out=xs0[:, k, :], in_=xT_src[:, k, 0:512])

            wqk_sb = p_w.tile([128, KD, NM * 128], BF16, tag="wqk")
            wqk_src = wqk_d.rearrange("(c p) n -> p c n", p=128)
            for k in range(KD):
                nc.sync.dma_start(out=wqk_sb[:, k, :], in_=wqk_src[:, k, :])

            wv_sb = p_w.tile([128, KD, NH * 64], BF16, tag="wv")
            wv_src = wv_d.rearrange("(c p) n -> p c n", p=128)
            for k in range(KD):
                nc.sync.dma_start(out=wv_sb[:, k, :], in_=wv_src[:, k, :])

            mask_sb = p_w.tile([128, 128], BF16, tag="mask")
            nc.sync.dma_start(out=mask_sb[:, :], in_=mask_d[:, :])

            wproj_sb = p_w.tile([128, NPAIR, D], BF16, tag="wproj")
            nc.sync.dma_start(out=wproj_sb[:, :, :], in_=wproj_d[:, :, :])

            qkT_sb = p_qkt.tile([128, NM, S], BF16, tag="qkt")
            v_aug = p_vaug.tile([128, NSK, NH, 65], BF16, tag="vaug")
            nc.vector.memset(v_aug[:, :, :, 64:65], 1.0)

            def load_xs(j):
                xs = p_stream.tile([128, KD, 512], BF16, tag="xs")
                for k in range(KD):
                    nc.sync.dma_start(
                        out=xs[:, k, :],
                        in_=xT_src[:, k, j * 512:(j + 1) * 512],
                    )
                return xs

            def qk_part(j, xs, mp):
                ps_qk = p_misc.tile([128, NB], F32, tag="m")
                for k in range(KD):
                    nc.tensor.matmul(
                        ps_qk[:, :],
                        lhsT=wqk_sb[:, k, mp * 128:(mp + 1) * 128],
                        rhs=xs[:, k, :],
                        start=(k == 0),
                        stop=(k == KD - 1),
                    )
                nc.vector.tensor_scalar_add(
                    qkT_sb[:, mp, j * 512:(j + 1) * 512],
                    ps_qk[:, :],
                    bqk_sb[:, mp:mp + 1],
                )

            def v_part(j, xs, m):
                ps_v = p_misc.tile([128, NB], F32, tag="m")
                for k in range(KD):
                    nc.tensor.matmul(
                        ps_v[:, 0:NH * 64],
                        lhsT=xs[:, k, (m % 4) * 128:(m % 4) * 128 + 128],
                        rhs=wv_sb[:, k, :],
                        start=(k == 0),
                        stop=(k == KD - 1),
                    )
                nc.vector.tensor_add(
                    v_aug[:, m, :, 0:64],
                    ps_v[:, 0:NH * 64].rearrange("p (h c) -> p h c", c=64),
                    bvbc_sb[:, :].rearrange("p (h c) -> p h c", c=64),
                )

            def qkv_parts(j, xs):
                parts = []
                for mp in range(NM):
                    parts.append(lambda mp=mp: qk_part(j, xs, mp))
                for m in range(4 * j, 4 * j + 4):
                    parts.append(lambda m=m: v_part(j, xs, m))
                return parts

            attn_T = {}

            def attention_block(j, fillers=()):
                fillers = list(fillers)
                attn_T[j] = p_attn_T.tile([128, NPAIR, 512], BF16, tag="attnT", name=f"attnT{j}")
                attn_n = None
                for h in range(NH):
                    member, pair = h % 2, h // 2
                    qT = qkT_sb[64 * member:64 * member + 64, h // 2, :]
                    kT = qkT_sb[64 * member:64 * member + 64, NMQ + h // 2, :]
                    if member == 0:
                        attn_n = p_attn_n.tile([128, 4, 128], BF16, tag="attnn")
                    ps_attn = p_av.tile([128, NH, 65], F32, tag="av")
                    npair = 2 * (j + 1)

                    def emit_scores(g):
                        # pair of sk tiles i=2g, 2g+1; exact causal column
                        # truncation (bf16 keeps 1 cycle/row at any N)
                        ps = p_s.tile([128, 2, 512], F32, tag="s")
                        for b in range(2):
                            i = 2 * g + b
                            no = 128 * max(0, i - 4 * j)
                            nc.tensor.matmul(
                                ps[:, b, no:512],
                                lhsT=kT[:, i * 128:(i + 1) * 128],
                                rhs=qT[:, j * 512 + no:(j + 1) * 512],
                                start=True,
                                stop=True,
                            )
                        return ps

                    sc_next = emit_scores(0)
                    for g in range(npair):
                        ps_sc = sc_next
                        # 1-deep software pipeline: next group's scores are
                        # emitted before this group's AV so PE runs them
                        # while ACT computes this group's exp
                        if g + 1 < npair:
                            sc_next = emit_scores(g + 1)
                        exp_t = p_exp.tile([128, 2, 512], BF16, tag="exp")
                        if g == 2 * j:
                            # diagonal pair 1: b0 full, b1 valid >= 128 (its
                            # cols 0:128 read stale PSUM; never consumed)
                            nc.scalar.activation(
                                exp_t[:, :, :], ps_sc[:, :, :], AFT.Exp,
                                scale=SCALE,
                            )
                        elif g == 2 * j + 1:
                            # diagonal pair 2: b0 valid >= 256, b1 >= 384
                            # (b1 cols 256:384 stale; never consumed)
                            nc.scalar.activation(
                                exp_t[:, :, 256:512],
                                ps_sc[:, :, 256:512],
                                AFT.Exp,
                                scale=SCALE,
                            )
                        else:
                            nc.scalar.activation(
                                exp_t[:, :, :], ps_sc[:, :, :], AFT.Exp,
                                scale=SCALE,
                            )
                        if g >= 2 * j:
                            # in-tile causal mask of diagonal tiles, off the
                            # DVE/ACT path (Pool is otherwise idle)
                            for b in range(2):
                                c = 2 * g + b - 4 * j
                                nc.gpsimd.tensor_mul(
                                    exp_t[:, b, 128 * c:128 * c + 128],
                                    exp_t[:, b, 128 * c:128 * c + 128],
                                    mask_sb[:, :],
                                )
                        for b in range(2):
                            i = 2 * g + b
                            clo = max(0, i - 4 * j)
                            for c in range(clo, 4):
                                nc.tensor.matmul(
                                    ps_attn[:, c, :],
                                    lhsT=exp_t[:, b, 128 * c:128 * c + 128],
                                    rhs=v_aug[:, i, h, :],
                                    start=(i == 0),
                                    stop=(i == 4 * j + c),
                                )
                    # normalize: denominators live per-partition (sq) here,
                    # so a [128,1]-scalar broadcast along the free dim works
                    rc = p_rc.tile([128, NH], F32, tag="rc")
                    nc.vector.reciprocal(
                        rc[:, :],
                        ps_attn[:, :, 64:65].rearrange("p a b -> p (a b)"),
                    )
                    rc_ap = rc[:, :]
                    rc_bc = bass.AP(
                        tensor=rc_ap.tensor,
                        offset=rc_ap.offset,
                        ap=list(rc_ap.ap) + [[0, 64]],
                    )
                    nc.vector.tensor_mul(
                        attn_n[:, :, 64 * member:64 * member + 64],
                        ps_attn[:, :, 0:64],
                        rc_bc,
                    )
                    if member == 1:
                        # XBAR transpose [sq 128, 2 heads x 64] -> [128, sq]:
                        # partitions become the paired head dim for proj
                        for c in range(4):
                            nc.sync.dma_start_transpose(
                                out=attn_T[j][:, pair, 128 * c:128 * c + 128],
                                in_=attn_n[:, c, :],
                            )
                    # drain PE filler work into the ACT-paced stretch
                    if h >= 1:
                        take = max(1, (len(fillers) + NH - 1 - h) // (NH - h))
                        for _ in range(take):
                            if fillers:
                                fillers.pop(0)()
                for f in fillers:
                    f()

            def proj_part(j, m):
                o = (m % 4) * 128
                for n in range(ND):
                    ps_y = p_misc.tile([128, NB], F32, tag="m")
                    for p in range(NPAIR):
                        nc.tensor.matmul(
                            ps_y[:, :],
                            lhsT=attn_T[j][:, p, o:o + 128],
                            rhs=wproj_sb[:, p, n * NB:(n + 1) * NB],
                            start=(p == 0),
                            stop=(p == NPAIR - 1),
                        )
                    y_sb = p_y.tile([128, NB], BF16, tag="y")
                    nc.vector.tensor_copy(y_sb[:, :], ps_y[:, :])
                    nc.sync.dma_start(
                        out=y_d[m * 128:(m + 1) * 128, n * NB:(n + 1) * NB],
                        in_=y_sb[:, :],
                    )

            def proj_parts(j):
                return [
                    (lambda m=m: proj_part(j, m)) for m in range(j * 4, j * 4 + 4)
                ]

            # j=0 prologue: qkv computed up front. The qk parts come first
            # (their weights land first); junk matmuls bridge the wv-DMA wait
            parts0 = qkv_parts(0, xs0)
            for part in parts0[:NM]:
                part()
            for _w in range(40):
                nc.tensor.matmul(
                    warm_ps[0:64, 0, 0:64],
                    lhsT=ones_sb[:, :],
                    rhs=ones_sb[:, :],
                    start=True,
                    stop=True,
                )
            for part in parts0[NM:]:
                part()
            xs_next = load_xs(1) if SQB > 1 else None
            for j in range(SQB):
                fillers = []
                if j + 1 < SQB:
                    fillers += qkv_parts(j + 1, xs_next)
                    xs_after = load_xs(j + 2) if j + 2 < SQB else None
                else:
                    xs_after = None
                if j >= 1:
                    fillers += proj_parts(j - 1)
                attention_block(j, fillers)
                xs_next = xs_after
            for part in proj_parts(SQB - 1):
                part()

    return nc


def make_mask():
    p = np.arange(128)[:, None]
    f = np.arange(128)[None, :]
    return (f >= p)  # [128, 128] valid region in T layout


def make_in_maps(x, W_qkv, b_qkv, W_proj):
    """Per-core input dicts for the full-size problem."""
    import ml_dtypes

    bf = ml_dtypes.bfloat16
    masku = make_mask().astype(bf)
    in_maps = []
    for c in range(NCORES):
        b, q = c // 4, c % 4
        cq = slice(256 * q, 256 * q + 256)
        wqk = np.concatenate([W_qkv[:, cq], W_qkv[:, 1024:2048][:, cq]], axis=1)
        wv = W_qkv[:, 2048:3072][:, cq]
        bqk = np.concatenate([b_qkv[cq], b_qkv[1024:2048][cq]]).reshape(4, 128)
        bvbc = np.broadcast_to(b_qkv[2048:3072][cq], (128, 256))
        wproj = np.ascontiguousarray(
            W_proj[cq, :].reshape(2, 128, 1024).transpose(1, 0, 2)
        )
        in_maps.append(
            {
                "xT": np.ascontiguousarray(x[b].T).astype(bf),
                "wqk": np.ascontiguousarray(wqk).astype(bf),
                "wv": np.ascontiguousarray(wv).astype(bf),
                "bqk": np.ascontiguousarray(bqk),
                "bvbc": np.ascontiguousarray(bvbc),
                "wproj": wproj.astype(bf),
                "masku": masku,
            }
        )
    return in_maps


_NC_CACHE = {}


def _get_nc():
    if "nc" not in _NC_CACHE:
        _NC_CACHE["nc"] = build_nc()
    return _NC_CACHE["nc"]


def run_on_hw(x, W_qkv, b_qkv, W_proj, b_proj, trace=False, **trace_kw):
    from concourse.bass_utils import run_bass_kernel_spmd

    in_maps = make_in_maps(x, W_qkv, b_qkv, W_proj)
    res = run_bass_kernel_spmd(
        _get_nc(), in_maps, core_ids=list(range(NCORES)), trace=trace, **trace_kw
    )
    out = np.empty((B, S, D), dtype=np.float32)
    for b in range(B):
        acc = res.results[4 * b]["y"].astype(np.float32)
        for q in range(1, 4):
            acc = acc + res.results[4 * b + q]["y"].astype(np.float32)
        out[b] = acc + b_proj[None, :]
    return out, res


def kernel(x, W_qkv, b_qkv, W_proj, b_proj):
    x = np.asarray(x, dtype=np.float32)
    W_qkv = np.asarray(W_qkv, dtype=np.float32)
    b_qkv = np.asarray(b_qkv, dtype=np.float32)
    W_proj = np.asarray(W_proj, dtype=np.float32)
    b_proj = np.asarray(b_proj, dtype=np.float32)
    out, _ = run_on_hw(x, W_qkv, b_qkv, W_proj, b_proj, trace=False)
    return out


# revision 11
# speedup vs baseline: 1.1273x; 1.0300x over previous
"""Trainium2 Bass kernel for causal multi-head self-attention.

Problem (hardcoded):
    x:      [2, 2048, 1024] f32
    W_qkv:  [1024, 3072] f32   (cols: [q | k | v], each 1024 = 16 heads x 64)
    b_qkv:  [3072] f32
    W_proj: [1024, 1024] f32
    b_proj: [1024] f32
    out:    [2, 2048, 1024] f32

Sharding over 8 NeuronCores: data parallel on batch (2) x tensor parallel on
heads (4 quads of 4 heads). Core c handles batch c//4, heads [4*(c%4), 4*(c%4)+4).
Each core computes its heads' q/k/v projections, causal-softmax attention, and a
partial output projection (its heads' rows of W_proj). Host gather sums the 4
bf16 partials per batch in f32 and adds b_proj.

On-core dataflow (bf16 operands, f32 PSUM accumulation):
  - qkT [512, S]   = W_qk^T @ x^T   (partitions = qkv-col; 4 M-tiles of 128)
  - v_aug [S,4,65] = x @ W_v + ones column per head (softmax denominator)
  - scoresT[sk,sq] = kT.T @ qT per head, exact 128-granular causal truncation
  - expT = ACT exp(0.125 * scores) -> bf16 (no max-subtraction: |s/8| small)
  - diagonal tiles masked on GpSimd (Pool) with a [128,128] triangular mask
  - AV transposed: ps_attn[sq 128, 4 chunks, 65] += expT_chunk^T @ v_aug
    (N=65 per accumulation step: half the PE cycles of the [65, sq] form)
  - normalize per-partition (sq) via DVE reciprocal + broadcast multiply
  - head pair packed side by side [sq, 2*64] -> XBAR DMA transpose -> [128, sq]
  - y tile [sq 128, 512] = sum over 2 pairs: attn_T^T @ W_proj_pair (K=128)
"""

import os
import sys

for _p in ("/opt/trn_rl_repo", "/root/.axon_site/_ro/trn_rl_repo"):
    if os.path.isdir(_p) and _p not in sys.path:
        sys.path.append(_p)

import numpy as np

import concourse.bass as bass
import concourse.mybir as mybir
import concourse.tile as tile
from concourse import library_config

F32 = mybir.dt.float32
BF16 = mybir.dt.bfloat16
AFT = mybir.ActivationFunctionType

B, S, D, H, HD = 2, 2048, 1024, 16, 64
NCORES = 8
NH = 4  # heads per core
SCALE = 1.0 / 8.0  # 1/sqrt(64)


class SplitWaitTileContext(tile.TileContext):
    """This container's walrus rejects >1 sync wait per instruction
    ("Too many sync wait commands"). Split extra waits onto preceding
    same-engine NoOps before the final block lowering."""

    def _lower_ordered_insts(self, ordered):
        for bb_name, insts in list(ordered.items()):
            new = []
            for inst in insts:
                si = inst.sync_info
                if si is not None and si.on_wait and len(si.on_wait) > 1:
                    waits = list(si.on_wait)
                    for w in waits[:-1]:
                        nop = mybir.InstNoOp(
                            name=f"nopw-{self.nc.get_next_instruction_name()}"
                        )
                        nop.engine = inst.engine
                        nop.sync_info = mybir.SyncInfo(on_wait=[w], on_update=[])
                        new.append(nop)
                    inst.sync_info = mybir.SyncInfo(
                        on_wait=[waits[-1]], on_update=list(si.on_update or [])
                    )
                new.append(inst)
            ordered[bb_name] = new
        return super()._lower_ordered_insts(ordered)

    def _drain_and_barrier(self, tick_clock, wait_clock):
        from concourse.vector_clock import ScopedClock

        drain_inst = self.nc.sync.drain()
        wait_clock.add_sem_waits(
            drain_inst.ins, ScopedClock({None: tick_clock.global_clock})
        )
        si = drain_inst.ins.sync_info
        if si is not None and si.on_wait and len(si.on_wait) > 1:
            waits = list(si.on_wait)
            drain_inst.ins.sync_info = mybir.SyncInfo(
                on_wait=[waits[0]], on_update=list(si.on_update or [])
            )
            for w in waits[1:]:
                nop = self.nc.sync.nop(nofuse=True)
                nop.ins.sync_info = mybir.SyncInfo(on_wait=[w], on_update=[])

        self.nc.all_engine_barrier()
        assert self.sems is not None
        popped = self.nc._tile_sem_poison_stack.pop()
        assert popped is self._sem_poison
        self.nc.clear_and_free_semaphores(list(self.sems.allocated().values()))
        self.nc.all_engine_barrier()


def build_nc(S=S, D=D, NH=NH, dbg=False, reps=1):
    """Build the single-core SPMD program."""
    KD = D // 128        # k-chunks of the D contraction
    NM = NH              # qk M-tiles: 2 q tiles then 2 k tiles
    NMQ = NM // 2
    SQB = S // 512       # sq blocks of 512
    NSK = S // 128       # sk tiles of 128
    NB = min(512, D)     # proj output column block size
    ND = D // NB         # proj output column blocks
    NPAIR = NH // 2

    nc = bass.Bass("TRN2", target_bir_lowering=False, debug=False)

    xT_d = nc.dram_tensor("xT", [D, S], BF16, kind="ExternalInput").ap()
    wqk_d = nc.dram_tensor("wqk", [D, NM * 128], BF16, kind="ExternalInput").ap()
    wv_d = nc.dram_tensor("wv", [D, NH * 64], BF16, kind="ExternalInput").ap()
    bqk_d = nc.dram_tensor("bqk", [NM, 128], F32, kind="ExternalInput").ap()
    bvbc_d = nc.dram_tensor("bvbc", [128, NH * 64], F32, kind="ExternalInput").ap()
    wproj_d = nc.dram_tensor(
        "wproj", [128, NPAIR, D], BF16, kind="ExternalInput"
    ).ap()
    mask_d = nc.dram_tensor("masku", [128, 128], BF16, kind="ExternalInput").ap()
    y_d = nc.dram_tensor("y", [S, D], BF16, kind="ExternalOutput").ap()

    with SplitWaitTileContext(nc) as tc:
        with (
            nc.allow_low_precision(reason="bf16 operands; fp32 accum in PSUM"),
            tc.tile_pool(name="stream", bufs=4) as p_stream,
            tc.tile_pool(name="wpool", bufs=1) as p_w,
            tc.tile_pool(name="qkt", bufs=1) as p_qkt,
            tc.tile_pool(name="vaug", bufs=1) as p_vaug,
            tc.tile_pool(name="expp", bufs=4) as p_exp,
            tc.tile_pool(name="attnn", bufs=2) as p_attn_n,
            tc.tile_pool(name="attnT", bufs=4) as p_attn_T,
            tc.tile_pool(name="rcp", bufs=2) as p_rc,
            tc.tile_pool(name="ypool", bufs=4) as p_y,
            tc.tile_pool(name="pmisc", bufs=2, space="PSUM") as p_misc,
            tc.tile_pool(name="ps", bufs=2, space="PSUM") as p_s,
            tc.tile_pool(name="pav", bufs=2, space="PSUM") as p_av,
        ):
          for _rep in range(reps):
            # PE warmup: junk matmuls keep the systolic array ramped while
            # the input DMAs land
            ones_sb = p_w.tile([128, 64], BF16, tag="ones")
            nc.vector.memset(ones_sb[:, :], 1.0)
            # preload the exp table set in the startup window
            expwarm = p_w.tile([1, 1], F32, tag="expwarm")
            nc.scalar.activation(
                expwarm[:, :], ones_sb[0:1, 0:1], AFT.Exp, scale=SCALE
            )
            warm_ps = p_av.tile([128, NH, 65], F32, tag="av")
            for _w in range(40):
                nc.tensor.matmul(
                    warm_ps[0:64, 0, 0:64],
                    lhsT=ones_sb[:, :],
                    rhs=ones_sb[:, :],
                    start=True,
                    stop=True,
                )

            # biases are tiny but gate the qkT adds: issue them first
            bqk_sb = p_w.tile([128, NM], F32, tag="bqk")
            nc.sync.dma_start(out=bqk_sb[:, :], in_=bqk_d.rearrange("m p -> p m"))
            bvbc_sb = p_w.tile([128, NH * 64], F32, tag="bvbc")
            nc.sync.dma_start(out=bvbc_sb[:, :], in_=bvbc_d[:, :])

            # xT column 0 + qk weights first (split in halves so the first
            # matmuls start while the second halves land); batched DMAs keep
            # the serialized HWDGE issue path short
            xT_src = xT_d.rearrange("(c p) s -> p c s", p=128)
            xs0 = p_stream.tile([128, KD, 512], BF16, tag="xs", name="xs0")
            nc.sync.dma_start(out=xs0[:, 0:KD // 2, :], in_=xT_src[:, 0:KD // 2, 0:512])
            nc.sync.dma_start(out=xs0[:, KD // 2:, :], in_=xT_src[:, KD // 2:, 0:512])

            wqk_sb = p_w.tile([128, KD, NM * 128], BF16, tag="wqk")
            wqk_src = wqk_d.rearrange("(c p) n -> p c n", p=128)
            nc.sync.dma_start(out=wqk_sb[:, 0:KD // 2, :], in_=wqk_src[:, 0:KD // 2, :])
            nc.sync.dma_start(out=wqk_sb[:, KD // 2:, :], in_=wqk_src[:, KD // 2:, :])

            wv_sb = p_w.tile([128, KD, NH * 64], BF16, tag="wv")
            wv_src = wv_d.rearrange("(c p) n -> p c n", p=128)
            nc.sync.dma_start(out=wv_sb[:, :, :], in_=wv_src[:, :, :])

            mask_sb = p_w.tile([128, 128], BF16, tag="mask")
            nc.sync.dma_start(out=mask_sb[:, :], in_=mask_d[:, :])

            qkT_sb = p_qkt.tile([128, NM, S], BF16, tag="qkt")
            v_aug = p_vaug.tile([128, NSK, NH, 65], BF16, tag="vaug")
            nc.vector.memset(v_aug[:, :, :, 64:65], 1.0)

            def load_xs(j):
                xs = p_stream.tile([128, KD, 512], BF16, tag="xs", name=f"xs{j}")
                nc.sync.dma_start(
                    out=xs[:, :, :],
                    in_=xT_src[:, :, j * 512:(j + 1) * 512],
                )
                return xs

            # prefetch the remaining x blocks + proj weights up front: SBUF
            # is plentiful and this keeps qkv fillers off the DMA wait path
            xs_all = {0: xs0}
            for j in range(1, SQB):
                xs_all[j] = load_xs(j)

            wproj_sb = p_w.tile([128, NPAIR, D], BF16, tag="wproj")
            nc.sync.dma_start(out=wproj_sb[:, :, :], in_=wproj_d[:, :, :])

            def qk_part(j, xs, mp):
                ps_qk = p_misc.tile([128, NB], F32, tag="m")
                for k in range(KD):
                    nc.tensor.matmul(
                        ps_qk[:, :],
                        lhsT=wqk_sb[:, k, mp * 128:(mp + 1) * 128],
                        rhs=xs[:, k, :],
                        start=(k == 0),
                        stop=(k == KD - 1),
                    )
                nc.vector.tensor_scalar_add(
                    qkT_sb[:, mp, j * 512:(j + 1) * 512],
                    ps_qk[:, :],
                    bqk_sb[:, mp:mp + 1],
                )

            def v_part(j, xs, m):
                ps_v = p_misc.tile([128, NB], F32, tag="m")
                for k in range(KD):
                    nc.tensor.matmul(
                        ps_v[:, 0:NH * 64],
                        lhsT=xs[:, k, (m % 4) * 128:(m % 4) * 128 + 128],
                        rhs=wv_sb[:, k, :],
                        start=(k == 0),
                        stop=(k == KD - 1),
                    )
                nc.vector.tensor_add(
                    v_aug[:, m, :, 0:64],
                    ps_v[:, 0:NH * 64].rearrange("p (h c) -> p h c", c=64),
                    bvbc_sb[:, :].rearrange("p (h c) -> p h c", c=64),
                )

            def qkv_parts(j, xs):
                parts = []
                for mp in range(NM):
                    parts.append(lambda mp=mp: qk_part(j, xs, mp))
                for m in range(4 * j, 4 * j + 4):
                    parts.append(lambda m=m: v_part(j, xs, m))
                return parts

            attn_T = {}

            def attention_block(j, fillers=()):
                fillers = list(fillers)
                attn_T[j] = p_attn_T.tile([128, NPAIR, 512], BF16, tag="attnT", name=f"attnT{j}")
                attn_n = None
                for h in range(NH):
                    member, pair = h % 2, h // 2
                    qT = qkT_sb[64 * member:64 * member + 64, h // 2, :]
                    kT = qkT_sb[64 * member:64 * member + 64, NMQ + h // 2, :]
                    if member == 0:
                        attn_n = p_attn_n.tile([128, 4, 128], BF16, tag="attnn")
                    ps_attn = p_av.tile([128, NH, 65], F32, tag="av")
                    npair = 2 * (j + 1)

                    def emit_scores(g):
                        # pair of sk tiles i=2g, 2g+1; exact causal column
                        # truncation (bf16 keeps 1 cycle/row at any N)
                        ps = p_s.tile([128, 2, 512], F32, tag="s")
                        for b in range(2):
                            i = 2 * g + b
                            no = 128 * max(0, i - 4 * j)
                            nc.tensor.matmul(
                                ps[:, b, no:512],
                                lhsT=kT[:, i * 128:(i + 1) * 128],
                                rhs=qT[:, j * 512 + no:(j + 1) * 512],
                                start=True,
                                stop=True,
                            )
                        return ps

                    sc_next = emit_scores(0)
                    for g in range(npair):
                        ps_sc = sc_next
                        # 1-deep software pipeline: next group's scores are
                        # emitted before this group's AV so PE runs them
                        # while ACT computes this group's exp
                        if g + 1 < npair:
                            sc_next = emit_scores(g + 1)
                        exp_t = p_exp.tile([128, 2, 512], BF16, tag="exp")
                        if g == 2 * j:
                            # diagonal pair 1: b0 full, b1 valid >= 128 (its
                            # cols 0:128 read stale PSUM; never consumed)
                            nc.scalar.activation(
                                exp_t[:, :, :], ps_sc[:, :, :], AFT.Exp,
                                scale=SCALE,
                            )
                        elif g == 2 * j + 1:
                            # diagonal pair 2: b0 valid >= 256, b1 >= 384
                            # (b1 cols 256:384 stale; never consumed)
                            nc.scalar.activation(
                                exp_t[:, :, 256:512],
                                ps_sc[:, :, 256:512],
                                AFT.Exp,
                                scale=SCALE,
                            )
                        else:
                            nc.scalar.activation(
                                exp_t[:, :, :], ps_sc[:, :, :], AFT.Exp,
                                scale=SCALE,
                            )
                        if g >= 2 * j:
                            # in-tile causal mask of diagonal tiles, off the
                            # DVE/ACT path (Pool is otherwise idle)
                            for b in range(2):
                                c = 2 * g + b - 4 * j
                                nc.gpsimd.tensor_mul(
                                    exp_t[:, b, 128 * c:128 * c + 128],
                                    exp_t[:, b, 128 * c:128 * c + 128],
                                    mask_sb[:, :],
                                )
                        for b in range(2):
                            i = 2 * g + b
                            clo = max(0, i - 4 * j)
                            for c in range(clo, 4):
                                nc.tensor.matmul(
                                    ps_attn[:, c, :],
                                    lhsT=exp_t[:, b, 128 * c:128 * c + 128],
                                    rhs=v_aug[:, i, h, :],
                                    start=(i == 0),
                                    stop=(i == 4 * j + c),
                                )
                    # normalize: denominators live per-partition (sq) here,
                    # so a [128,1]-scalar broadcast along the free dim works
                    rc = p_rc.tile([128, NH], F32, tag="rc")
                    nc.vector.reciprocal(
                        rc[:, :],
                        ps_attn[:, :, 64:65].rearrange("p a b -> p (a b)"),
                    )
                    rc_ap = rc[:, :]
                    rc_bc = bass.AP(
                        tensor=rc_ap.tensor,
                        offset=rc_ap.offset,
                        ap=list(rc_ap.ap) + [[0, 64]],
                    )
                    nc.vector.tensor_mul(
                        attn_n[:, :, 64 * member:64 * member + 64],
                        ps_attn[:, :, 0:64],
                        rc_bc,
                    )
                    if member == 1:
                        # XBAR transpose [sq 128, 2 heads x 64] -> [128, sq]:
                        # partitions become the paired head dim for proj.
                        # One blocked-transpose instruction flips all 4
                        # chunks: out[:, c, :] = in[:, 128c:128c+128].T
                        nc.sync.dma_start_transpose(
                            out=attn_T[j][:, pair, :].rearrange(
                                "p (c f) -> p c f", f=128
                            ),
                            in_=attn_n[:, :, :],
                        )
                    # drain PE filler work into the ACT-paced stretch
                    take = max(1, (len(fillers) + NH - 1 - h) // (NH - h))
                    for _ in range(take):
                        if fillers:
                            fillers.pop(0)()
                for f in fillers:
                    f()

            def proj_part(j, m):
                o = (m % 4) * 128
                y_sb = p_y.tile([128, D], BF16, tag="y")
                for n in range(ND):
                    ps_y = p_misc.tile([128, NB], F32, tag="m")
                    for p in range(NPAIR):
                        nc.tensor.matmul(
                            ps_y[:, :],
                            lhsT=attn_T[j][:, p, o:o + 128],
                            rhs=wproj_sb[:, p, n * NB:(n + 1) * NB],
                            start=(p == 0),
                            stop=(p == NPAIR - 1),
                        )
                    nc.vector.tensor_copy(y_sb[:, n * NB:(n + 1) * NB], ps_y[:, :])
                nc.sync.dma_start(
                    out=y_d[m * 128:(m + 1) * 128, :],
                    in_=y_sb[:, :],
                )

            def proj_parts(j):
                return [
                    (lambda m=m: proj_part(j, m)) for m in range(j * 4, j * 4 + 4)
                ]

            # j=0 prologue: qkv computed up front. The qk parts come first
            # (their weights land first); junk matmuls bridge the wv-DMA wait
            parts0 = qkv_parts(0, xs0)
            for part in parts0[:NM]:
                part()
            for _w in range(40):
                nc.tensor.matmul(
                    warm_ps[0:64, 0, 0:64],
                    lhsT=ones_sb[:, :],
                    rhs=ones_sb[:, :],
                    start=True,
                    stop=True,
                )
            for part in parts0[NM:]:
                part()
            # proj fillers are pushed toward the late (ACT-heavy) blocks:
            # the exp load grows with j while qkv filler supply is constant
            proj_sched = {2: [0], 3: [1, 2]} if SQB == 4 else {
                j: [j - 1] for j in range(1, SQB)
            }
            for j in range(SQB):
                fillers = []
                if j + 1 < SQB:
                    fillers += qkv_parts(j + 1, xs_all[j + 1])
                for jp in proj_sched.get(j, []):
                    fillers += proj_parts(jp)
                attention_block(j, fillers)
            for part in proj_parts(SQB - 1):
                part()

    return nc


def make_mask():
    p = np.arange(128)[:, None]
    f = np.arange(128)[None, :]
    return (f >= p)  # [128, 128] valid region in T layout


def make_in_maps(x, W_qkv, b_qkv, W_proj):
    """Per-core input dicts for the full-size problem."""
    import ml_dtypes

    bf = ml_dtypes.bfloat16
    masku = make_mask().astype(bf)
    in_maps = []
    for c in range(NCORES):
        b, q = c // 4, c % 4
        cq = slice(256 * q, 256 * q + 256)
        wqk = np.concatenate([W_qkv[:, cq], W_qkv[:, 1024:2048][:, cq]], axis=1)
        wv = W_qkv[:, 2048:3072][:, cq]
        bqk = np.concatenate([b_qkv[cq], b_qkv[1024:2048][cq]]).reshape(4, 128)
        bvbc = np.broadcast_to(b_qkv[2048:3072][cq], (128, 256))
        wproj = np.ascontiguousarray(
            W_proj[cq, :].reshape(2, 128, 1024).transpose(1, 0, 2)
        )
        in_maps.append(
            {
                "xT": np.ascontiguousarray(x[b].T).astype(bf),
                "wqk": np.ascontiguousarray(wqk).astype(bf),
                "wv": np.ascontiguousarray(wv).astype(bf),
                "bqk": np.ascontiguousarray(bqk),
                "bvbc": np.ascontiguousarray(bvbc),
                "wproj": wproj.astype(bf),
                "masku": masku,
            }
        )
    return in_maps


_NC_CACHE = {}


def _get_nc():
    if "nc" not in _NC_CACHE:
        _NC_CACHE["nc"] = build_nc()
    return _NC_CACHE["nc"]


def run_on_hw(x, W_qkv, b_qkv, W_proj, b_proj, trace=False, **trace_kw):
    from concourse.bass_utils import run_bass_kernel_spmd

    in_maps = make_in_maps(x, W_qkv, b_qkv, W_proj)
    res = run_bass_kernel_spmd(
        _get_nc(), in_maps, core_ids=list(range(NCORES)), trace=trace, **trace_kw
    )
    out = np.empty((B, S, D), dtype=np.float32)
    for b in range(B):
        acc = res.results[4 * b]["y"].astype(np.float32)
        for q in range(1, 4):
            acc = acc + res.results[4 * b + q]["y"].astype(np.float32)
        out[b] = acc + b_proj[None, :]
    return out, res


def kernel(x, W_qkv, b_qkv, W_proj, b_proj):
    x = np.asarray(x, dtype=np.float32)
    W_qkv = np.asarray(W_qkv, dtype=np.float32)
    b_qkv = np.asarray(b_qkv, dtype=np.float32)
    W_proj = np.asarray(W_proj, dtype=np.float32)
    b_proj = np.asarray(b_proj, dtype=np.float32)
    out, _ = run_on_hw(x, W_qkv, b_qkv, W_proj, b_proj, trace=False)
    return out


# revision 17
# speedup vs baseline: 1.2029x; 1.0671x over previous
"""Trainium2 Bass kernel for causal multi-head self-attention.

Problem (hardcoded):
    x:      [2, 2048, 1024] f32
    W_qkv:  [1024, 3072] f32   (cols: [q | k | v], each 1024 = 16 heads x 64)
    b_qkv:  [3072] f32
    W_proj: [1024, 1024] f32
    b_proj: [1024] f32
    out:    [2, 2048, 1024] f32

Sharding over 8 NeuronCores: data parallel on batch (2) x tensor parallel on
heads (4 quads of 4 heads). Core c handles batch c//4, heads [4*(c%4), 4*(c%4)+4).
Each core computes its heads' q/k/v projections, causal-softmax attention, and a
partial output projection (its heads' rows of W_proj). Host gather sums the 4
bf16 partials per batch in f32 and adds b_proj.

On-core dataflow (bf16 operands, f32 PSUM accumulation):
  - qkT [512, S]   = W_qk^T @ x^T   (partitions = qkv-col; 4 M-tiles of 128)
  - v_aug [S,4,65] = x @ W_v + ones column per head (softmax denominator)
  - scoresT[sk,sq] = kT.T @ qT per head, exact 128-granular causal truncation
  - expT = ACT exp(0.125 * scores) -> bf16 (no max-subtraction: |s/8| small)
  - diagonal tiles masked on GpSimd (Pool) with a [128,128] triangular mask
  - AV transposed: ps_attn[sq 128, 4 chunks, 65] += expT_chunk^T @ v_aug
    (N=65 per accumulation step: half the PE cycles of the [65, sq] form)
  - normalize per-partition (sq) via DVE reciprocal + broadcast multiply
  - head pair packed side by side [sq, 2*64] -> XBAR DMA transpose -> [128, sq]
  - y tile [sq 128, 512] = sum over 2 pairs: attn_T^T @ W_proj_pair (K=128)
"""

import os
import sys

for _p in ("/opt/trn_rl_repo", "/root/.axon_site/_ro/trn_rl_repo"):
    if os.path.isdir(_p) and _p not in sys.path:
        sys.path.append(_p)

import numpy as np

import concourse.bass as bass
import concourse.mybir as mybir
import concourse.tile as tile
from concourse import library_config

F32 = mybir.dt.float32
BF16 = mybir.dt.bfloat16
AFT = mybir.ActivationFunctionType

B, S, D, H, HD = 2, 2048, 1024, 16, 64
NCORES = 8
NH = 4  # heads per core
SCALE = 1.0 / 8.0  # 1/sqrt(64)


class SplitWaitTileContext(tile.TileContext):
    """This container's walrus rejects >1 sync wait per instruction
    ("Too many sync wait commands"). Split extra waits onto preceding
    same-engine NoOps before the final block lowering."""

    def _lower_ordered_insts(self, ordered):
        for bb_name, insts in list(ordered.items()):
            new = []
            for inst in insts:
                si = inst.sync_info
                if si is not None and si.on_wait and len(si.on_wait) > 1:
                    waits = list(si.on_wait)
                    for w in waits[:-1]:
                        nop = mybir.InstNoOp(
                            name=f"nopw-{self.nc.get_next_instruction_name()}"
                        )
                        nop.engine = inst.engine
                        nop.sync_info = mybir.SyncInfo(on_wait=[w], on_update=[])
                        new.append(nop)
                    inst.sync_info = mybir.SyncInfo(
                        on_wait=[waits[-1]], on_update=list(si.on_update or [])
                    )
                new.append(inst)
            ordered[bb_name] = new
        return super()._lower_ordered_insts(ordered)

    def _drain_and_barrier(self, tick_clock, wait_clock):
        from concourse.vector_clock import ScopedClock

        drain_inst = self.nc.sync.drain()
        wait_clock.add_sem_waits(
            drain_inst.ins, ScopedClock({None: tick_clock.global_clock})
        )
        si = drain_inst.ins.sync_info
        if si is not None and si.on_wait and len(si.on_wait) > 1:
            waits = list(si.on_wait)
            drain_inst.ins.sync_info = mybir.SyncInfo(
                on_wait=[waits[0]], on_update=list(si.on_update or [])
            )
            for w in waits[1:]:
                nop = self.nc.sync.nop(nofuse=True)
                nop.ins.sync_info = mybir.SyncInfo(on_wait=[w], on_update=[])

        self.nc.all_engine_barrier()
        assert self.sems is not None
        popped = self.nc._tile_sem_poison_stack.pop()
        assert popped is self._sem_poison
        self.nc.clear_and_free_semaphores(list(self.sems.allocated().values()))
        self.nc.all_engine_barrier()


def build_nc(S=S, D=D, NH=NH, dbg=False, reps=1):
    """Build the single-core SPMD program."""
    KD = D // 128        # k-chunks of the D contraction
    NM = NH              # qk M-tiles: 2 q tiles then 2 k tiles
    NMQ = NM // 2
    SQB = S // 512       # sq blocks of 512
    NSK = S // 128       # sk tiles of 128
    NB = min(512, D)     # proj output column block size
    ND = D // NB         # proj output column blocks
    NPAIR = NH // 2

    nc = bass.Bass("TRN2", target_bir_lowering=False, debug=False)

    xT_d = nc.dram_tensor("xT", [D, S], BF16, kind="ExternalInput").ap()
    wqk_d = nc.dram_tensor("wqk", [D, NM * 128], BF16, kind="ExternalInput").ap()
    wv_d = nc.dram_tensor("wv", [D, NH * 64], BF16, kind="ExternalInput").ap()
    bqk_d = nc.dram_tensor("bqk", [NM, 128], F32, kind="ExternalInput").ap()
    bvbc_d = nc.dram_tensor("bvbc", [128, NH * 64], F32, kind="ExternalInput").ap()
    wproj_d = nc.dram_tensor(
        "wproj", [128, NPAIR, D], BF16, kind="ExternalInput"
    ).ap()
    mask_d = nc.dram_tensor("masku", [128, 128], BF16, kind="ExternalInput").ap()
    y_d = nc.dram_tensor("y", [S, D], BF16, kind="ExternalOutput").ap()

    with SplitWaitTileContext(nc) as tc:
        with (
            nc.allow_low_precision(reason="bf16 operands; fp32 accum in PSUM"),
            tc.tile_pool(name="stream", bufs=4) as p_stream,
            tc.tile_pool(name="wpool", bufs=1) as p_w,
            tc.tile_pool(name="qkt", bufs=1) as p_qkt,
            tc.tile_pool(name="vaug", bufs=1) as p_vaug,
            tc.tile_pool(name="expp", bufs=4) as p_exp,
            tc.tile_pool(name="attnn", bufs=2) as p_attn_n,
            tc.tile_pool(name="attnT", bufs=4) as p_attn_T,
            tc.tile_pool(name="rcp", bufs=2) as p_rc,
            tc.tile_pool(name="ypool", bufs=4) as p_y,
            tc.tile_pool(name="pmisc", bufs=2, space="PSUM") as p_misc,
            tc.tile_pool(name="ps", bufs=2, space="PSUM") as p_s,
            tc.tile_pool(name="pav", bufs=2, space="PSUM") as p_av,
        ):
          for _rep in range(reps):
            # PE warmup: junk matmuls keep the systolic array ramped while
            # the input DMAs land
            ones_sb = p_w.tile([128, 64], BF16, tag="ones")
            nc.vector.memset(ones_sb[:, :], 1.0)
            # preload the exp table set in the startup window
            expwarm = p_w.tile([1, 1], F32, tag="expwarm")
            nc.scalar.activation(
                expwarm[:, :], ones_sb[0:1, 0:1], AFT.Exp, scale=SCALE
            )
            warm_ps = p_av.tile([128, NH, 65], F32, tag="av")
            for _w in range(40):
                nc.tensor.matmul(
                    warm_ps[0:64, 0, 0:64],
                    lhsT=ones_sb[:, :],
                    rhs=ones_sb[:, :],
                    start=True,
                    stop=True,
                )

            # startup DMA order: everything the first qk matmuls need first
            # (halves so the first matmuls start while second halves land);
            # batched DMAs keep the serialized HWDGE issue path short
            bqk_sb = p_w.tile([128, NM], F32, tag="bqk")
            nc.sync.dma_start(out=bqk_sb[:, :], in_=bqk_d.rearrange("m p -> p m"))

            xT_src = xT_d.rearrange("(c p) s -> p c s", p=128)
            xs0 = p_stream.tile([128, KD, 512], BF16, tag="xs", name="xs0")
            wqk_sb = p_w.tile([128, KD, NM * 128], BF16, tag="wqk")
            wqk_src = wqk_d.rearrange("(c p) n -> p c n", p=128)
            nc.sync.dma_start(out=xs0[:, 0:KD // 2, :], in_=xT_src[:, 0:KD // 2, 0:512])
            nc.sync.dma_start(out=wqk_sb[:, 0:KD // 2, :], in_=wqk_src[:, 0:KD // 2, :])
            nc.sync.dma_start(out=xs0[:, KD // 2:, :], in_=xT_src[:, KD // 2:, 0:512])
            nc.sync.dma_start(out=wqk_sb[:, KD // 2:, :], in_=wqk_src[:, KD // 2:, :])

            wv_sb = p_w.tile([128, KD, NH * 64], BF16, tag="wv")
            wv_src = wv_d.rearrange("(c p) n -> p c n", p=128)
            nc.sync.dma_start(out=wv_sb[:, :, :], in_=wv_src[:, :, :])

            bvbc_sb = p_w.tile([128, NH * 64], F32, tag="bvbc")
            nc.sync.dma_start(out=bvbc_sb[:, :], in_=bvbc_d[:, :])

            mask_sb = p_w.tile([128, 128], BF16, tag="mask")
            nc.sync.dma_start(out=mask_sb[:, :], in_=mask_d[:, :])

            qkT_sb = p_qkt.tile([128, NM, S], BF16, tag="qkt")
            v_aug = p_vaug.tile([128, NSK, NH, 65], BF16, tag="vaug")
            nc.vector.memset(v_aug[:, :, :, 64:65], 1.0)

            def load_xs(j):
                xs = p_stream.tile([128, KD, 512], BF16, tag="xs", name=f"xs{j}")
                nc.sync.dma_start(
                    out=xs[:, :, :],
                    in_=xT_src[:, :, j * 512:(j + 1) * 512],
                )
                return xs

            # prefetch the remaining x blocks + proj weights up front: SBUF
            # is plentiful and this keeps qkv fillers off the DMA wait path
            xs_all = {0: xs0}
            for j in range(1, SQB):
                xs_all[j] = load_xs(j)

            wproj_sb = p_w.tile([128, NPAIR, D], BF16, tag="wproj")
            nc.sync.dma_start(out=wproj_sb[:, :, :], in_=wproj_d[:, :, :])

            def qk_part(j, xs, mp):
                ps_qk = p_misc.tile([128, NB], F32, tag="m")
                for k in range(KD):
                    nc.tensor.matmul(
                        ps_qk[:, :],
                        lhsT=wqk_sb[:, k, mp * 128:(mp + 1) * 128],
                        rhs=xs[:, k, :],
                        start=(k == 0),
                        stop=(k == KD - 1),
                    )
                nc.vector.tensor_scalar_add(
                    qkT_sb[:, mp, j * 512:(j + 1) * 512],
                    ps_qk[:, :],
                    bqk_sb[:, mp:mp + 1],
                )

            def v_part(j, xs, m):
                ps_v = p_misc.tile([128, NB], F32, tag="m")
                for k in range(KD):
                    nc.tensor.matmul(
                        ps_v[:, 0:NH * 64],
                        lhsT=xs[:, k, (m % 4) * 128:(m % 4) * 128 + 128],
                        rhs=wv_sb[:, k, :],
                        start=(k == 0),
                        stop=(k == KD - 1),
                    )
                nc.vector.tensor_add(
                    v_aug[:, m, :, 0:64],
                    ps_v[:, 0:NH * 64].rearrange("p (h c) -> p h c", c=64),
                    bvbc_sb[:, :].rearrange("p (h c) -> p h c", c=64),
                )

            def qkv_parts(j, xs):
                parts = []
                for mp in range(NM):
                    parts.append(lambda mp=mp: qk_part(j, xs, mp))
                for m in range(4 * j, 4 * j + 4):
                    parts.append(lambda m=m: v_part(j, xs, m))
                return parts

            attn_T = {}

            def attention_block(j, fillers=()):
                fillers = list(fillers)
                n_fill = len(fillers)
                total_groups = NH * 2 * (j + 1)
                groups_done = 0
                popped = 0

                def pump():
                    # spread fillers evenly over the block's exp groups;
                    # emitted BEFORE each group's AV so the (in-order) PE
                    # works on them while ACT computes the group's exp
                    nonlocal groups_done, popped
                    groups_done += 1
                    want = (n_fill * groups_done) // total_groups
                    while popped < want and fillers:
                        fillers.pop(0)()
                        popped += 1

                attn_T[j] = p_attn_T.tile([128, NPAIR, 512], BF16, tag="attnT", name=f"attnT{j}")
                attn_n = None
                for h in range(NH):
                    member, pair = h % 2, h // 2
                    qT = qkT_sb[64 * member:64 * member + 64, h // 2, :]
                    kT = qkT_sb[64 * member:64 * member + 64, NMQ + h // 2, :]
                    if member == 0:
                        attn_n = p_attn_n.tile([128, 4, 128], BF16, tag="attnn")
                    ps_attn = p_av.tile([128, NH, 65], F32, tag="av")
                    npair = 2 * (j + 1)

                    def emit_scores(g):
                        # pair of sk tiles i=2g, 2g+1; exact causal column
                        # truncation (bf16 keeps 1 cycle/row at any N)
                        ps = p_s.tile([128, 2, 512], F32, tag="s")
                        for b in range(2):
                            i = 2 * g + b
                            no = 128 * max(0, i - 4 * j)
                            nc.tensor.matmul(
                                ps[:, b, no:512],
                                lhsT=kT[:, i * 128:(i + 1) * 128],
                                rhs=qT[:, j * 512 + no:(j + 1) * 512],
                                start=True,
                                stop=True,
                            )
                        return ps

                    sc_next = emit_scores(0)
                    for g in range(npair):
                        ps_sc = sc_next
                        # 1-deep software pipeline: next group's scores are
                        # emitted before this group's AV so PE runs them
                        # while ACT computes this group's exp
                        if g + 1 < npair:
                            sc_next = emit_scores(g + 1)
                        exp_t = p_exp.tile([128, 2, 512], BF16, tag="exp")
                        if g == 2 * j:
                            # diagonal pair 1: b0 full, b1 valid >= 128 (its
                            # cols 0:128 read stale PSUM; never consumed)
                            nc.scalar.activation(
                                exp_t[:, :, :], ps_sc[:, :, :], AFT.Exp,
                                scale=SCALE,
                            )
                        elif g == 2 * j + 1:
                            # diagonal pair 2: b0 valid >= 256, b1 >= 384
                            # (b1 cols 256:384 stale; never consumed)
                            nc.scalar.activation(
                                exp_t[:, :, 256:512],
                                ps_sc[:, :, 256:512],
                                AFT.Exp,
                                scale=SCALE,
                            )
                        else:
                            nc.scalar.activation(
                                exp_t[:, :, :], ps_sc[:, :, :], AFT.Exp,
                                scale=SCALE,
                            )
                        if g >= 2 * j:
                            # in-tile causal mask of diagonal tiles, off the
                            # DVE/ACT path (Pool is otherwise idle)
                            for b in range(2):
                                c = 2 * g + b - 4 * j
                                nc.gpsimd.tensor_mul(
                                    exp_t[:, b, 128 * c:128 * c + 128],
                                    exp_t[:, b, 128 * c:128 * c + 128],
                                    mask_sb[:, :],
                                )
                        pump()
                        for b in range(2):
                            i = 2 * g + b
                            clo = max(0, i - 4 * j)
                            # diagonal (masked) chunk last: its AV also waits
                            # on the Pool mask and PE executes in order
                            for c in list(range(clo + 1, 4)) + [clo]:
                                nc.tensor.matmul(
                                    ps_attn[:, c, :],
                                    lhsT=exp_t[:, b, 128 * c:128 * c + 128],
                                    rhs=v_aug[:, i, h, :],
                                    start=(i == 0),
                                    stop=(i == 4 * j + c),
                                )
                    # normalize: denominators live per-partition (sq) here,
                    # so a [128,1]-scalar broadcast along the free dim works
                    rc = p_rc.tile([128, NH], F32, tag="rc")
                    nc.vector.reciprocal(
                        rc[:, :],
                        ps_attn[:, :, 64:65].rearrange("p a b -> p (a b)"),
                    )
                    rc_ap = rc[:, :]
                    rc_bc = bass.AP(
                        tensor=rc_ap.tensor,
                        offset=rc_ap.offset,
                        ap=list(rc_ap.ap) + [[0, 64]],
                    )
                    nc.vector.tensor_mul(
                        attn_n[:, :, 64 * member:64 * member + 64],
                        ps_attn[:, :, 0:64],
                        rc_bc,
                    )
                    if member == 1:
                        # XBAR transpose [sq 128, 2 heads x 64] -> [128, sq]:
                        # partitions become the paired head dim for proj.
                        # One blocked-transpose instruction flips all 4
                        # chunks: out[:, c, :] = in[:, 128c:128c+128].T
                        nc.sync.dma_start_transpose(
                            out=attn_T[j][:, pair, :].rearrange(
                                "p (c f) -> p c f", f=128
                            ),
                            in_=attn_n[:, :, :],
                        )
                for f in fillers:
                    f()

            def proj_part(j, m):
                o = (m % 4) * 128
                y_sb = p_y.tile([128, D], BF16, tag="y")
                for n in range(ND):
                    ps_y = p_misc.tile([128, NB], F32, tag="m")
                    for p in range(NPAIR):
                        nc.tensor.matmul(
                            ps_y[:, :],
                            lhsT=attn_T[j][:, p, o:o + 128],
                            rhs=wproj_sb[:, p, n * NB:(n + 1) * NB],
                            start=(p == 0),
                            stop=(p == NPAIR - 1),
                        )
                    nc.vector.tensor_copy(y_sb[:, n * NB:(n + 1) * NB], ps_y[:, :])
                nc.sync.dma_start(
                    out=y_d[m * 128:(m + 1) * 128, :],
                    in_=y_sb[:, :],
                )

            def proj_parts(j):
                return [
                    (lambda m=m: proj_part(j, m)) for m in range(j * 4, j * 4 + 4)
                ]

            # j=0 prologue: qkv computed up front. The qk parts come first
            # (their weights land first); junk matmuls bridge the wv-DMA wait
            parts0 = qkv_parts(0, xs0)
            for part in parts0[:NM]:
                part()
            for _w in range(40):
                nc.tensor.matmul(
                    warm_ps[0:64, 0, 0:64],
                    lhsT=ones_sb[:, :],
                    rhs=ones_sb[:, :],
                    start=True,
                    stop=True,
                )
            for part in parts0[NM:]:
                part()
            # proj fillers are pushed toward the last (ACT-heaviest) block:
            # the exp load grows with j while qkv filler supply is constant
            proj_sched = {3: [0, 1, 2]} if SQB == 4 else {
                j: [j - 1] for j in range(1, SQB)
            }
            for j in range(SQB):
                fillers = []
                if j + 1 < SQB:
                    fillers += qkv_parts(j + 1, xs_all[j + 1])
                for jp in proj_sched.get(j, []):
                    fillers += proj_parts(jp)
                attention_block(j, fillers)
            for part in proj_parts(SQB - 1):
                part()

    return nc


def make_mask():
    p = np.arange(128)[:, None]
    f = np.arange(128)[None, :]
    return (f >= p)  # [128, 128] valid region in T layout


def make_in_maps(x, W_qkv, b_qkv, W_proj):
    """Per-core input dicts for the full-size problem."""
    import ml_dtypes

    bf = ml_dtypes.bfloat16
    masku = make_mask().astype(bf)
    in_maps = []
    for c in range(NCORES):
        b, q = c // 4, c % 4
        cq = slice(256 * q, 256 * q + 256)
        wqk = np.concatenate([W_qkv[:, cq], W_qkv[:, 1024:2048][:, cq]], axis=1)
        wv = W_qkv[:, 2048:3072][:, cq]
        bqk = np.concatenate([b_qkv[cq], b_qkv[1024:2048][cq]]).reshape(4, 128)
        bvbc = np.broadcast_to(b_qkv[2048:3072][cq], (128, 256))
        wproj = np.ascontiguousarray(
            W_proj[cq, :].reshape(2, 128, 1024).transpose(1, 0, 2)
        )
        in_maps.append(
            {
                "xT": np.ascontiguousarray(x[b].T).astype(bf),
                "wqk": np.ascontiguousarray(wqk).astype(bf),
                "wv": np.ascontiguousarray(wv).astype(bf),
                "bqk": np.ascontiguousarray(bqk),
                "bvbc": np.ascontiguousarray(bvbc),
                "wproj": wproj.astype(bf),
                "masku": masku,
            }
        )
    return in_maps


_NC_CACHE = {}


def _get_nc():
    if "nc" not in _NC_CACHE:
        _NC_CACHE["nc"] = build_nc()
    return _NC_CACHE["nc"]


def run_on_hw(x, W_qkv, b_qkv, W_proj, b_proj, trace=False, **trace_kw):
    from concourse.bass_utils import run_bass_kernel_spmd

    in_maps = make_in_maps(x, W_qkv, b_qkv, W_proj)
    res = run_bass_kernel_spmd(
        _get_nc(), in_maps, core_ids=list(range(NCORES)), trace=trace, **trace_kw
    )
    out = np.empty((B, S, D), dtype=np.float32)
    for b in range(B):
        acc = res.results[4 * b]["y"].astype(np.float32)
        for q in range(1, 4):
            acc = acc + res.results[4 * b + q]["y"].astype(np.float32)
        out[b] = acc + b_proj[None, :]
    return out, res


def kernel(x, W_qkv, b_qkv, W_proj, b_proj):
    x = np.asarray(x, dtype=np.float32)
    W_qkv = np.asarray(W_qkv, dtype=np.float32)
    b_qkv = np.asarray(b_qkv, dtype=np.float32)
    W_proj = np.asarray(W_proj, dtype=np.float32)
    b_proj = np.asarray(b_proj, dtype=np.float32)
    out, _ = run_on_hw(x, W_qkv, b_qkv, W_proj, b_proj, trace=False)
    return out
